# revision 34
# baseline (speedup 1.0000x reference)
"""Self-contained Trainium2 Bass kernel for a 3-stage dense GAT + linear head.

Row-parallel across 8 NeuronCores: core c owns output rows [c*512, (c+1)*512).

Math: GAT scores are a rank-1 outer sum e_ij = f1_i + f2_j, so
exp(leakyrelu(e)) factors per branch:
  s>0:  exp(f1_i) * exp(f2_j)          s<=0: exp(.2 f1_i) * exp(.2 f2_j)
With the 0/1 selector A_ij = adj_ij * [f1_i + f2_j > 0] and per-j scaled
extended rows  uext_j = [u*Wh | u | v*Wh | v]  (u = exp(f2), v = exp(.2 f2)):
  h_i ~ eu_i * (A @ [uWh|u]) + ev_i * ((adj - A) @ [vWh|v])
(the adj-sum minus A-sum gives the negative-branch complement; softmax
denominators come from the appended u/v columns).  This is EXACT — the u/v
weights live in the matmul rhs, so the only N^2 elementwise work is one 4x-mode
is_lt compare per (j-tile, head) plus one 2x-mode mask multiply per (j-tile,
group) on VectorE; everything else is TensorE matmuls.

Distribution: each core builds the extended rows for its OWN nodes only and
an AllGather (split in 2 chunks so gather overlaps attention) shares them;
scores never materialize (they live as 0/1 selectors in [j_part, i_free]
layout and are consumed immediately by TensorE accumulation).
Stage-1 rows depend only on kernel inputs, so the host precomputes them in
fp32 and the device starts directly with the attention loop.
"""

import numpy as np

N = 4096
F0 = 512
H = 4
NCLASS = 40
NCORES = 8
R = N // NCORES          # 512 rows per core
IC = R // 128            # 4 i-chunks of 128
NT = N // 128            # 32 j-tiles of 128
NTO = R // 128           # own j-tiles per core
NCH = 2                  # allgather chunks per stage hand-off
TPC = NTO // NCH         # own j-tiles per chunk
STAGES = [
    # (Fin, O, head_groups)
    (512, 64, [(0, 1), (2, 3)]),
    (256, 32, [(0, 1, 2, 3)]),
    (128, 16, [(0, 1, 2, 3)]),
]
# Heads whose indicator is built on ScalarE as sign(f1+f2) in {-1,0,1}
# (DVE is the bottleneck engine; Sign shares the natural_log_exp_and_others
# activation table with Exp/Ln/Copy, so no table reloads). A sign-valued
# selector needs the full-D mask-sum: S_A = (pa'' + pm)/2.
ACT_HEADS = (0,)

_CACHE = {}


def _ext_cols(O):
    # [uWh(0:O) | u(O) | vWh(E:E+O) | v(D-1) | f2(D) | pad(D+1)]
    E = O + 1
    D = 2 * E
    return E, D, D + 2


def _tile_order():
    """Global j-tile processing order for gathered stages: chunk 0 tiles of
    every core first, then chunk 1 — matches allgather chunk arrival."""
    order = []
    for k in range(NCH):
        for r in range(NCORES):
            for l in range(TPC):
                order.append(r * NTO + k * TPC + l)
    return order


def _build(single=False, reps=1, ablate=()):
    import concourse.bacc as bacc
    import concourse.mybir as mybir
    import concourse.tile as tile

    dt = mybir.dt
    AF = mybir.ActivationFunctionType
    OP = mybir.AluOpType
    X = mybir.AxisListType.X

    nc = bacc.Bacc("TRN2", target_bir_lowering=False, debug=False,
                   num_devices=1 if single else NCORES)

    E0, D0, W0 = _ext_cols(STAGES[0][1])

    # ---- I/O ----
    adjT = nc.dram_tensor("adjT", [N, R], dt.bfloat16, kind="ExternalInput")
    uext0_d = nc.dram_tensor("uext0", [N, H * W0], dt.bfloat16,
                             kind="ExternalInput")
    f1neg0_d = nc.dram_tensor("f1neg0", [1, H * R], dt.bfloat16,
                              kind="ExternalInput")
    eu0_d = nc.dram_tensor("eu0", [R, H], dt.float32, kind="ExternalInput")
    ev0_d = nc.dram_tensor("ev0", [R, H], dt.float32, kind="ExternalInput")
    wcat_d = {}
    for s, (Fin, O, _) in enumerate(STAGES):
        if s == 0:
            continue
        # [W concat by head | W@a_dst (H cols) | W@a_src (H cols)]
        wcat_d[s] = nc.dram_tensor(f"W{s}cat", [Fin, H * O + 2 * H],
                                   dt.bfloat16, kind="ExternalInput")
    ident_d = nc.dram_tensor("ident", [128, 128], dt.bfloat16,
                             kind="ExternalInput")
    wlin_d = nc.dram_tensor("wlin", [H * STAGES[2][1], NCLASS], dt.bfloat16,
                            kind="ExternalInput")
    blin_d = nc.dram_tensor("blin", [1, NCLASS], dt.float32, kind="ExternalInput")
    out_d = nc.dram_tensor("out_blk", [R, NCLASS], dt.float32,
                           kind="ExternalOutput")

    # ---- internal DRAM (stage hand-off + collectives) ----
    RC = R // NCH                      # own rows per chunk
    ccin_d, ccout_d = {}, {}
    for s, (Fin, O, _) in enumerate(STAGES):
        if s < 2:
            _, _, Wn = _ext_cols(STAGES[s + 1][1])
            ccin_d[s] = nc.dram_tensor(f"ccin{s}", [R, H * Wn], dt.bfloat16,
                                       kind="Internal")
            for k in range(NCH):
                ccout_d[(s, k)] = nc.dram_tensor(
                    f"ccout{s}_{k}", [NCORES * RC, H * Wn], dt.bfloat16,
                    kind="Internal", addr_space="Shared")

    order = _tile_order()

    with tile.TileContext(nc) as tc:
        with (
            tc.tile_pool(name="glob", bufs=1) as gp,
            tc.tile_pool(name="work", bufs=4) as wp,
            tc.tile_pool(name="small", bufs=2) as sp,
            tc.tile_pool(name="psum", bufs=1, space="PSUM") as pp,
            tc.tile_pool(name="psum2", bufs=2, space="PSUM") as pp2,
        ):
            ones_bf = gp.tile([1, 128], dt.bfloat16, tag="ones_bf")
            nc.gpsimd.memset(ones_bf[:], 1.0)
            ones_f = gp.tile([1, 128], dt.float32, tag="ones_f")
            nc.gpsimd.memset(ones_f[:], 1.0)

            ACC_W = 396  # per-i-chunk PSUM bank: G*(2E) A-sums + G*E m-sums

            # All input-dependent loads live INSIDE the rep loop so each rep
            # is a complete execution (full HBM traffic) — reps>1 exists only
            # to amortize NEFF launch overhead when timing.
            for rep in range(reps):
              # small stage-1 dependencies first so attention starts early
              f1n_sb = gp.tile([1, H, R], dt.bfloat16, tag="f1n_sb")
              nc.scalar.dma_start(f1n_sb[:], f1neg0_d[:].rearrange(
                  "q (h r) -> q h r", h=H))
              eu = gp.tile([128, IC, H], dt.float32, tag="eu")
              ev = gp.tile([128, IC, H], dt.float32, tag="ev")
              nc.scalar.dma_start(
                  eu[:], eu0_d[:].rearrange("(i p) h -> p i h", p=128))
              nc.scalar.dma_start(
                  ev[:], ev0_d[:].rearrange("(i p) h -> p i h", p=128))

              # stage-1 ext rows (host-built) on sync queue, masks on the ACT
              # HWDGE queue — interleaved per tile so tile t's deps co-arrive
              uwx0 = [gp.tile([128, H, W0], dt.bfloat16, tag="uwx0", bufs=NT,
                              name=f"uwx0_{t}") for t in range(NT)]
              mask = [gp.tile([128, R], dt.bfloat16, tag="mask", bufs=NT,
                              name=f"mk_{t}") for t in range(NT)]
              for t in range(NT):
                  nc.sync.dma_start(
                      uwx0[t][:],
                      uext0_d[t * 128:(t + 1) * 128, :].rearrange(
                          "p (h w) -> p h w", h=H))
                  nc.gpsimd.dma_start(mask[t][:], adjT[t * 128:(t + 1) * 128, :])

              wcat_t = {}
              for s, (Fin, O, _) in enumerate(STAGES):
                  if s == 0:
                      continue
                  ft_n = Fin // 128
                  w = gp.tile([128, ft_n, H * O + 2 * H], dt.bfloat16,
                              tag=f"wcat{s}")
                  for ft in range(ft_n):
                      nc.scalar.dma_start(w[:, ft, :],
                                          wcat_d[s][ft * 128:(ft + 1) * 128, :])
                  wcat_t[s] = w
              ident = gp.tile([128, 128], dt.bfloat16, tag="ident")
              nc.scalar.dma_start(ident[:], ident_d[:])
              wlin_t = gp.tile([H * STAGES[2][1], NCLASS], dt.bfloat16,
                               tag="wlin")
              nc.scalar.dma_start(wlin_t[:], wlin_d[:])
              blin_t = gp.tile([1, NCLASS], dt.float32, tag="blin")
              nc.scalar.dma_start(blin_t[:], blin_d[:])

              hT_own = None
              for s, (Fin, O, groups) in enumerate(STAGES):
                  ft_n = Fin // 128
                  HO = H * O
                  E, D, Wd = _ext_cols(O)

                  f1b = gp.tile([128, H, R], dt.bfloat16, tag="f1b")
                  evn = gp.tile([128, IC, H], dt.float32, tag="evn")

                  if s == 0:
                      uwx = uwx0
                      tile_seq = list(range(NT))
                      for h in range(H):
                          f1bps = pp2.tile([128, R], dt.float32, tag="mm_ps",
                                           name="f1bps")
                          nc.tensor.matmul(f1bps[:], ones_bf[:],
                                           f1n_sb[:, h, :], start=True,
                                           stop=True)
                          nc.scalar.activation(f1b[:, h, :], f1bps[:], AF.Copy)
                  else:
                      eu = gp.tile([128, IC, H], dt.float32, tag="eu2",
                                   name=f"eu{s}")
                      ev = gp.tile([128, IC, H], dt.float32, tag="ev2",
                                   name=f"ev{s}")
                      # ---- own-rows ext build, chunked for overlap ----
                      uo = gp.tile([128, NTO, H, Wd], dt.bfloat16, tag="uo",
                                   name=f"uo{s}")
                      whs = gp.tile([128, NTO, H, O], dt.bfloat16, tag="whs",
                                    name=f"whs{s}")
                      f2c = sp.tile([128, NTO, H], dt.float32, tag="f2c")
                      for k in range(NCH):
                          for nt in range(k * TPC, (k + 1) * TPC):
                              ps = pp2.tile([128, HO + 2 * H], dt.float32,
                                            tag="mm_ps", name="wh_ps")
                              for ft in range(ft_n):
                                  nc.tensor.matmul(
                                      ps[:],
                                      hT_own[:, ft, nt * 128:(nt + 1) * 128],
                                      wcat_t[s][:, ft, :],
                                      start=(ft == 0), stop=(ft == ft_n - 1))
                              psv = ps[:, 0:HO].rearrange("p (h o) -> p h o",
                                                          h=H)
                              nc.scalar.activation(whs[:, nt, :, :], psv,
                                                   AF.Copy)
                              nc.scalar.activation(f2c[:, nt, :],
                                                   ps[:, HO:HO + H], AF.Copy)
                          cs = slice(k * TPC, (k + 1) * TPC)
                          nc.scalar.activation(uo[:, cs, :, D:D + 1],
                                               f2c[:, cs], AF.Copy)
                          nc.scalar.activation(uo[:, cs, :, O:O + 1],
                                               f2c[:, cs], AF.Exp)
                          nc.scalar.activation(uo[:, cs, :, D - 1:D],
                                               f2c[:, cs], AF.Exp, scale=0.2)
                          ub = uo[:, cs, :, O:O + 1].broadcast_to(
                              (128, TPC, H, O))
                          nc.vector.tensor_tensor(uo[:, cs, :, 0:O],
                                                  whs[:, cs], ub, OP.mult)
                          vb = uo[:, cs, :, D - 1:D].broadcast_to(
                              (128, TPC, H, O))
                          nc.vector.tensor_tensor(uo[:, cs, :, E:E + O],
                                                  whs[:, cs], vb, OP.mult)
                          nc.vector.memset(uo[:, cs, :, D + 1:D + 2], 0.0)
                          for t in range(k * TPC, (k + 1) * TPC):
                              nc.sync.dma_start(
                                  ccin_d[s - 1][t * 128:(t + 1) * 128, :],
                                  uo[:, t, :, :].rearrange("p h w -> p (h w)"))
                          if single or "noag" in ablate:
                              for c in range(NCORES):
                                  nc.sync.dma_start(
                                      ccout_d[(s - 1, k)][c * RC:(c + 1) * RC, :],
                                      ccin_d[s - 1][k * RC:(k + 1) * RC, :])
                          else:
                              nc.gpsimd.collective_compute(
                                  "AllGather", OP.bypass,
                                  replica_groups=[list(range(NCORES))],
                                  ins=[ccin_d[s - 1][k * RC:(k + 1) * RC, :]],
                                  outs=[ccout_d[(s - 1, k)][:]])

                      uwx = [None] * NT
                      tile_seq = order
                      for i, t in enumerate(order):
                          r, l = t // NTO, t % NTO
                          k, lk = l // TPC, l % TPC
                          uwx[t] = gp.tile([128, H, Wd], dt.bfloat16,
                                           tag="uwx", bufs=NT,
                                           name=f"uwx{s}_{t}")
                          row0 = r * RC + lk * 128
                          nc.sync.dma_start(
                              uwx[t][:],
                              ccout_d[(s - 1, k)][row0:row0 + 128, :].rearrange(
                                  "p (h w) -> p h w", h=H))

                      # ---- f1 (free layout, negated, broadcast over parts)
                      for h in range(H):
                          f1ps = pp2.tile([1, R], dt.float32, tag="mm_ps",
                                          name="f1ps")
                          for ft in range(ft_n):
                              nc.tensor.matmul(
                                  f1ps[:],
                                  wcat_t[s][:, ft, HO + H + h:HO + H + h + 1],
                                  hT_own[:, ft, :],
                                  start=(ft == 0), stop=(ft == ft_n - 1))
                          f1sb = sp.tile([1, R], dt.bfloat16, tag="f1_sb")
                          nc.scalar.copy(f1sb[:], f1ps[:])
                          f1bps = pp2.tile([128, R], dt.float32, tag="mm_ps",
                                           name="f1bps")
                          nc.tensor.matmul(f1bps[:], ones_bf[:], f1sb[:],
                                           start=True, stop=True)
                          nc.scalar.activation(f1b[:, h, :], f1bps[:], AF.Copy,
                                               scale=-1.0)
                      # ---- eu/ev for own i-chunks
                      f1pa = gp.tile([128, IC, H], dt.float32, tag="f1pa")
                      for ic in range(IC):
                          wops = pp2.tile([128, 2 * H], dt.float32, tag="mm_ps",
                                          name="wops")
                          for ft in range(ft_n):
                              nc.tensor.matmul(
                                  wops[:],
                                  hT_own[:, ft, ic * 128:(ic + 1) * 128],
                                  wcat_t[s][:, ft, HO:HO + 2 * H],
                                  start=(ft == 0), stop=(ft == ft_n - 1))
                          nc.scalar.activation(f1pa[:, ic, :], wops[:, H:2 * H],
                                               AF.Copy)
                      nc.scalar.activation(eu[:], f1pa[:], AF.Exp)
                      nc.scalar.activation(ev[:], f1pa[:], AF.Exp, scale=0.2)

                  nc.vector.tensor_scalar_mul(evn[:], ev[:], -1.0)
                  eu2 = gp.tile([128, IC, H], dt.float32, tag="eu2h")
                  nc.vector.tensor_scalar_mul(eu2[:], eu[:], 0.5)
                  ev2 = gp.tile([128, IC, H], dt.float32, tag="ev2h")
                  nc.vector.tensor_scalar_mul(ev2[:], ev[:], 0.5)
                  evn2 = gp.tile([128, IC, H], dt.float32, tag="evn2h")
                  nc.vector.tensor_scalar_mul(evn2[:], ev[:], -0.5)
                  # fp32 copies of the per-j f2 compare scalars (is_lt wants f32)
                  f2s = [sp.tile([128, H], dt.float32, tag="f2s", bufs=NT,
                                 name=f"f2s{s}_{t}") for t in range(NT)]
                  for t in tile_seq:
                      nc.scalar.activation(f2s[t][:], uwx[t][:, :, D], AF.Copy)

                  # ---- attention: fused compare*mask + matmul accumulation
                  hn_tiles = [gp.tile([128, HO], dt.bfloat16, tag=f"hn_{ic}",
                                      name=f"hn{s}_{ic}")
                              for ic in range(IC)]
                  for grp in groups:
                      G = len(grp)
                      # per-head mask-sum width: Act(sign) heads need full D
                      mw = [D if h in ACT_HEADS else E for h in grp]
                      moff = [G * D + sum(mw[:gi]) for gi in range(G)]
                      acc_w = G * D + sum(mw)
                      accs = [pp.tile([128, acc_w], dt.float32, tag=f"accAB_{ic}",
                                      name=f"acc{s}_{grp[0]}_{ic}")
                              for ic in range(IC)]
                      # contiguous m-matmul runs of equal per-head width
                      mruns = []
                      for gi, h in enumerate(grp):
                          if mruns and mw[gi] == mruns[-1][2]:
                              mruns[-1][1] += 1
                          else:
                              mruns.append([gi, gi + 1, mw[gi]])
                      if "novec" in ablate:
                          Astat = gp.tile([128, G, R], dt.bfloat16,
                                          tag="Astat", name=f"Astat{s}_{grp[0]}")
                          nc.vector.memset(Astat[:], 1.0)
                      for ti, nt in enumerate(tile_seq):
                          if "novec" in ablate:
                              A = Astat
                          else:
                              cInd = wp.tile([128, G, R], dt.bfloat16,
                                             tag="cInd", bufs=4)
                              for gi, h in enumerate(grp):
                                  if h in ACT_HEADS:
                                      nc.scalar.activation(
                                          cInd[:, gi, :], f1b[:, h, :],
                                          AF.Sign,
                                          bias=f2s[nt][:, h:h + 1],
                                          scale=-1.0)
                                  else:
                                      nc.vector.tensor_scalar(
                                          cInd[:, gi, :], f1b[:, h, :],
                                          f2s[nt][:, h:h + 1], None, OP.is_lt)
                              A = wp.tile([128, G, R], dt.bfloat16, tag="A",
                                          bufs=4)
                              mb_ = mask[nt][:, None, :].broadcast_to(
                                  (128, G, R))
                              nc.vector.tensor_tensor(A[:], cInd[:], mb_,
                                                      OP.mult)
                          for gi, h in enumerate(grp):
                              for ic in range(IC):
                                  nc.tensor.matmul(
                                      accs[ic][:, gi * D:(gi + 1) * D],
                                      A[:, gi, ic * 128:(ic + 1) * 128],
                                      uwx[nt][:, h, 0:D],
                                      start=(ti == 0), stop=(ti == NT - 1))
                          for g0, g1, w_ in mruns:
                              c0 = 0 if w_ == D else E
                              for ic in range(IC):
                                  nc.tensor.matmul(
                                      accs[ic][:, moff[g0]:
                                               moff[g0] + (g1 - g0) * w_],
                                      mask[nt][:, ic * 128:(ic + 1) * 128],
                                      uwx[nt][:, grp[0] + g0:grp[0] + g1,
                                              c0:D],
                                      start=(ti == 0), stop=(ti == NT - 1))

                      # ---- epilogue: h = elu((eu*Su + ev*(Mv - Sv)) / Z)
                      # DVE heads: S_A = pa (0/1 selector);
                      # Act heads:  S_A = (pa'' + pm)/2 (sign selector)
                      for ic in range(IC):
                          for gi, h in enumerate(grp):
                              act_h = h in ACT_HEADS
                              pa_u = accs[ic][:, gi * D:gi * D + E]
                              pa_v = accs[ic][:, gi * D + E:(gi + 1) * D]
                              d1 = sp.tile([128, E], dt.float32, tag="d1")
                              d2 = sp.tile([128, E], dt.float32, tag="d2")
                              d3 = sp.tile([128, E], dt.float32, tag="d3")
                              if act_h:
                                  pm_u = accs[ic][:, moff[gi]:moff[gi] + E]
                                  pm_v = accs[ic][:, moff[gi] + E:
                                                 moff[gi] + D]
                                  nc.scalar.activation(
                                      d1[:], pa_u, AF.Copy,
                                      scale=eu2[:, ic, h:h + 1])
                                  d0 = sp.tile([128, E], dt.float32, tag="d0")
                                  nc.vector.scalar_tensor_tensor(
                                      d0[:], pm_u, eu2[:, ic, h:h + 1], d1[:],
                                      OP.mult, OP.add)
                                  nc.vector.scalar_tensor_tensor(
                                      d2[:], pm_v, ev2[:, ic, h:h + 1], d0[:],
                                      OP.mult, OP.add)
                                  nc.vector.scalar_tensor_tensor(
                                      d3[:], pa_v, evn2[:, ic, h:h + 1], d2[:],
                                      OP.mult, OP.add)
                              else:
                                  pm = accs[ic][:, moff[gi]:moff[gi] + E]
                                  nc.scalar.activation(
                                      d1[:], pa_u, AF.Copy,
                                      scale=eu[:, ic, h:h + 1])
                                  nc.vector.scalar_tensor_tensor(
                                      d2[:], pm, ev[:, ic, h:h + 1], d1[:],
                                      OP.mult, OP.add)
                                  nc.vector.scalar_tensor_tensor(
                                      d3[:], pa_v, evn[:, ic, h:h + 1], d2[:],
                                      OP.mult, OP.add)
                              r = sp.tile([128, 1], dt.float32, tag="rZ")
                              nc.vector.reciprocal(r[:], d3[:, O:O + 1])
                              t0 = sp.tile([128, O], dt.float32, tag="t0")
                              nc.vector.tensor_scalar(t0[:], d3[:, 0:O], r[:],
                                                      0.0, OP.mult, OP.min)
                              t1 = sp.tile([128, O], dt.float32, tag="t1")
                              nc.vector.tensor_scalar(t1[:], d3[:, 0:O], r[:],
                                                      0.0, OP.mult, OP.max)
                              e0 = sp.tile([128, O], dt.float32, tag="e0")
                              nc.scalar.activation(e0[:], t0[:], AF.Exp)
                              nc.vector.scalar_tensor_tensor(
                                  hn_tiles[ic][:, h * O:(h + 1) * O], e0[:], 1.0,
                                  t1[:], OP.subtract, OP.add)

                  # ---- hand-off: PE-transpose own rows for next stage ----
                  if s < 2:
                      nft = HO // 128
                      hT_own = gp.tile([128, nft, R], dt.bfloat16, tag="hTown",
                                       name=f"hTown{s + 1}")
                      for ic in range(IC):
                          for ft in range(nft):
                              tp = pp2.tile([128, 128], dt.bfloat16,
                                            tag="mm_ps", name="tp_ps")
                              nc.tensor.transpose(
                                  tp[:], hn_tiles[ic][:, ft * 128:(ft + 1) * 128],
                                  ident[:])
                              nc.scalar.activation(
                                  hT_own[:, ft, ic * 128:(ic + 1) * 128], tp[:],
                                  AF.Copy)

              # ---- final linear + log_softmax ----
              F3 = H * STAGES[2][1]  # 64
              h3T = gp.tile([F3, R], dt.bfloat16, tag="h3T")
              for ic in range(IC):
                  tp = pp2.tile([128, 128], dt.bfloat16, tag="mm_ps",
                                name=f"tp3_{ic}")
                  nc.tensor.transpose(tp[:F3, :], hn_tiles[ic][:, 0:F3],
                                      ident[:])
                  nc.scalar.activation(h3T[:, ic * 128:(ic + 1) * 128],
                                       tp[:F3, :], AF.Copy)

              blb_ps = pp2.tile([128, NCLASS], dt.float32, tag="mm_ps",
                                name="blb_ps")
              nc.tensor.matmul(blb_ps[:], ones_f[:], blin_t[:], start=True,
                               stop=True)
              blb = gp.tile([128, NCLASS], dt.float32, tag="blb")
              nc.vector.tensor_copy(blb[:], blb_ps[:])

              for ic in range(IC):
                  lg_ps = pp2.tile([128, NCLASS], dt.float32, tag="mm_ps",
                                   name="lg_ps")
                  nc.tensor.matmul(lg_ps[:], h3T[:, ic * 128:(ic + 1) * 128],
                                   wlin_t[:], start=True, stop=True)
                  lg = sp.tile([128, NCLASS], dt.float32, tag="lg")
                  nc.vector.tensor_tensor(lg[:], lg_ps[:], blb[:], OP.add)
                  mx = sp.tile([128, 1], dt.float32, tag="mx")
                  nc.vector.tensor_reduce(mx[:], lg[:], axis=X, op=OP.max)
                  negmx = sp.tile([128, 1], dt.float32, tag="negmx")
                  nc.vector.tensor_scalar_mul(negmx[:], mx[:], -1.0)
                  ex = sp.tile([128, NCLASS], dt.float32, tag="ex")
                  se = sp.tile([128, 1], dt.float32, tag="se")
                  nc.scalar.activation(ex[:], lg[:], AF.Exp, bias=negmx[:],
                                       accum_out=se[:])
                  ln_t = sp.tile([128, 1], dt.float32, tag="ln_t")
                  nc.scalar.activation(ln_t[:], se[:], AF.Ln)
                  negln = sp.tile([128, 1], dt.float32, tag="negln")
                  nc.vector.tensor_scalar_mul(negln[:], ln_t[:], -1.0)
                  ov = sp.tile([128, NCLASS], dt.float32, tag="ov")
                  nc.vector.tensor_scalar(ov[:], lg[:], negmx[:], negln[:],
                                          OP.add, OP.add)
                  nc.sync.dma_start(out_d[ic * 128:(ic + 1) * 128, :], ov[:])

    nc.compile()
    return nc


def _get_nc():
    if "nc" not in _CACHE:
        _CACHE["nc"] = _build()
    return _CACHE["nc"]


def _prep_in_maps(x, adj, W1, a1, W2, a2, W3, a3, Wlin, blin):
    import ml_dtypes
    bf16 = ml_dtypes.bfloat16

    x = np.asarray(x, np.float32)
    # transpose adj once (BLAS-friendly), slice per-core column blocks below
    adjT_bf = np.ascontiguousarray(
        (np.asarray(adj) > 0).T).astype(bf16)

    Ws = [np.asarray(W1, np.float32), np.asarray(W2, np.float32),
          np.asarray(W3, np.float32)]
    As = [np.asarray(a1, np.float32), np.asarray(a2, np.float32),
          np.asarray(a3, np.float32)]

    # ---- host-side stage-1 prep (exact fp32, BLAS matmuls) ----
    O0 = STAGES[0][1]
    E0, D0, W0c = _ext_cols(O0)
    Wh1 = (x @ Ws[0].transpose(1, 0, 2).reshape(F0, H * O0)).reshape(
        N, H, O0)                                                # [N,H,O]
    f2_1 = np.einsum('nho,ho->nh', Wh1, As[0][:, O0:])
    f1_1 = np.einsum('nho,ho->nh', Wh1, As[0][:, :O0])
    u1 = np.exp(f2_1)
    v1 = np.exp(0.2 * f2_1)
    uext0 = np.zeros((N, H, W0c), np.float32)
    uext0[:, :, 0:O0] = u1[:, :, None] * Wh1
    uext0[:, :, O0] = u1
    uext0[:, :, E0:E0 + O0] = v1[:, :, None] * Wh1
    uext0[:, :, D0 - 1] = v1
    uext0[:, :, D0] = f2_1

    shared = {"uext0": np.ascontiguousarray(
        uext0.reshape(N, H * W0c)).astype(bf16)}
    for s, (Fin, O, _) in enumerate(STAGES):
        if s == 0:
            continue
        W = Ws[s]  # [H, Fin, O]
        a = As[s]  # [H, 2*O]
        wcat = W.transpose(1, 0, 2).reshape(Fin, H * O)
        wd = np.einsum('hfo,ho->fh', W, a[:, O:])   # W @ a_dst
        ws_ = np.einsum('hfo,ho->fh', W, a[:, :O])  # W @ a_src
        shared[f"W{s}cat"] = np.ascontiguousarray(
            np.concatenate([wcat, wd, ws_], axis=1)).astype(bf16)
    shared["ident"] = np.eye(128, dtype=np.float32).astype(bf16)
    shared["wlin"] = np.asarray(Wlin, np.float32).astype(bf16)
    shared["blin"] = np.asarray(blin, np.float32).reshape(1, NCLASS)

    in_maps = []
    for c in range(NCORES):
        rows = slice(c * R, (c + 1) * R)
        m = dict(shared)
        m["adjT"] = np.ascontiguousarray(adjT_bf[:, rows])
        m["f1neg0"] = np.ascontiguousarray(
            (-f1_1[rows, :]).T.reshape(1, H * R)).astype(bf16)
        m["eu0"] = np.ascontiguousarray(np.exp(f1_1[rows, :]))
        m["ev0"] = np.ascontiguousarray(np.exp(0.2 * f1_1[rows, :]))
        in_maps.append(m)
    return in_maps


def _make_runner(nc):
    """Cached shard_map'd executable (same lowering as
    concourse.bass2jax.run_bass_via_pjrt, traced once; repeated kernel()
    calls skip re-trace and only pay H2D + execute + D2H)."""
    import jax
    from jax.sharding import Mesh, PartitionSpec
    from jax.experimental.shard_map import shard_map
    import concourse.bass2jax as b2j
    import concourse.mybir as mybir

    b2j.install_neuronx_cc_hook()
    partition_name = (nc.partition_id_tensor.name
                      if nc.partition_id_tensor else None)
    in_names, out_names, out_avals = [], [], []
    for alloc in nc.m.functions[0].allocations:
        if not isinstance(alloc, mybir.MemoryLocationSet):
            continue
        name = alloc.memorylocations[0].name
        if alloc.kind == "ExternalInput":
            if name != partition_name:
                in_names.append(name)
        elif alloc.kind == "ExternalOutput":
            out_avals.append(jax.core.ShapedArray(
                tuple(alloc.tensor_shape), mybir.dt.np(alloc.dtype)))
            out_names.append(name)
    n_params = len(in_names)
    all_in = list(in_names) + list(out_names)
    if partition_name:
        all_in.append(partition_name)

    def _body(*args):
        operands = list(args)
        if partition_name:
            operands.append(b2j.partition_id_tensor())
        outs = b2j._bass_exec_p.bind(
            *operands, out_avals=tuple(out_avals), in_names=tuple(all_in),
            out_names=tuple(out_names), lowering_input_output_aliases=(),
            sim_require_finite=True, sim_require_nnan=True, nc=nc)
        return tuple(outs)

    devices = jax.devices()[:NCORES]
    mesh = Mesh(np.asarray(devices), ("core",))
    n_outs = len(out_names)
    in_specs = (PartitionSpec("core"),) * (n_params + n_outs)
    out_specs = (PartitionSpec("core"),) * n_outs
    fn = jax.jit(shard_map(_body, mesh=mesh, in_specs=in_specs,
                           out_specs=out_specs, check_rep=False),
                 keep_unused=True)
    zero_outs = [np.zeros((NCORES * av.shape[0], *av.shape[1:]), av.dtype)
                 for av in out_avals]

    def run(in_maps):
        concat_in = [np.concatenate([np.asarray(in_maps[c][name])
                                     for c in range(NCORES)], axis=0)
                     for name in in_names]
        outs = fn(*concat_in, *zero_outs)
        i = out_names.index("out_blk")
        return np.asarray(outs[i])  # [NCORES*R, NCLASS] row-concat

    return run


def kernel(x, adj, W1, a1, W2, a2, W3, a3, Wlin, blin):
    nc = _get_nc()
    if "run" not in _CACHE:
        _CACHE["run"] = _make_runner(nc)
    in_maps = _prep_in_maps(x, adj, W1, a1, W2, a2, W3, a3, Wlin, blin)
    out = _CACHE["run"](in_maps)
    return np.ascontiguousarray(out).astype(np.float32)


# revision 35
# speedup vs baseline: 1.0151x; 1.0151x over previous
"""Self-contained Trainium2 Bass kernel for a 3-stage dense GAT + linear head.

Row-parallel across 8 NeuronCores: core c owns output rows [c*512, (c+1)*512).

Math: GAT scores are a rank-1 outer sum e_ij = f1_i + f2_j, so
exp(leakyrelu(e)) factors per branch:
  s>0:  exp(f1_i) * exp(f2_j)          s<=0: exp(.2 f1_i) * exp(.2 f2_j)
With the 0/1 selector A_ij = adj_ij * [f1_i + f2_j > 0] and per-j scaled
extended rows  uext_j = [u*Wh | u | v*Wh | v]  (u = exp(f2), v = exp(.2 f2)):
  h_i ~ eu_i * (A @ [uWh|u]) + ev_i * ((adj - A) @ [vWh|v])
(the adj-sum minus A-sum gives the negative-branch complement; softmax
denominators come from the appended u/v columns).  This is EXACT — the u/v
weights live in the matmul rhs, so the only N^2 elementwise work is one 4x-mode
is_lt compare per (j-tile, head) plus one 2x-mode mask multiply per (j-tile,
group) on VectorE; everything else is TensorE matmuls.

Distribution: each core builds the extended rows for its OWN nodes only and
an AllGather (split in 2 chunks so gather overlaps attention) shares them;
scores never materialize (they live as 0/1 selectors in [j_part, i_free]
layout and are consumed immediately by TensorE accumulation).
Stage-1 rows depend only on kernel inputs, so the host precomputes them in
fp32 and the device starts directly with the attention loop.
"""

import numpy as np

N = 4096
F0 = 512
H = 4
NCLASS = 40
NCORES = 8
R = N // NCORES          # 512 rows per core
IC = R // 128            # 4 i-chunks of 128
NT = N // 128            # 32 j-tiles of 128
NTO = R // 128           # own j-tiles per core
NCH = 2                  # allgather chunks per stage hand-off
TPC = NTO // NCH         # own j-tiles per chunk
STAGES = [
    # (Fin, O, head_groups)
    (512, 64, [(0, 1), (2, 3)]),
    (256, 32, [(0, 1, 2, 3)]),
    (128, 16, [(0, 1, 2, 3)]),
]
# Per-stage heads whose indicator is built on ScalarE as sign(f1+f2) in
# {-1,0,1} (Sign shares the natural_log_exp_and_others activation table with
# Exp/Ln/Copy, so no table reloads). A sign-valued selector needs the full-D
# mask-sum (S_A = (pa'' + pm)/2), whose split m-matmul makes PE-burst-bound
# stage 1 worse — so the offload applies only in stages 2/3 where DVE is the
# clear bottleneck.
ACT_HEADS_BY_STAGE = ((), (0,), (0,))

_CACHE = {}


def _ext_cols(O):
    # [uWh(0:O) | u(O) | vWh(E:E+O) | v(D-1) | f2(D) | pad(D+1)]
    E = O + 1
    D = 2 * E
    return E, D, D + 2


def _tile_order():
    """Global j-tile processing order for gathered stages: chunk 0 tiles of
    every core first, then chunk 1 — matches allgather chunk arrival."""
    order = []
    for k in range(NCH):
        for r in range(NCORES):
            for l in range(TPC):
                order.append(r * NTO + k * TPC + l)
    return order


def _build(single=False, reps=1, ablate=()):
    import concourse.bacc as bacc
    import concourse.mybir as mybir
    import concourse.tile as tile

    dt = mybir.dt
    AF = mybir.ActivationFunctionType
    OP = mybir.AluOpType
    X = mybir.AxisListType.X

    nc = bacc.Bacc("TRN2", target_bir_lowering=False, debug=False,
                   num_devices=1 if single else NCORES)

    E0, D0, W0 = _ext_cols(STAGES[0][1])

    # ---- I/O ----
    adjT = nc.dram_tensor("adjT", [N, R], dt.bfloat16, kind="ExternalInput")
    uext0_d = nc.dram_tensor("uext0", [N, H * W0], dt.bfloat16,
                             kind="ExternalInput")
    f1neg0_d = nc.dram_tensor("f1neg0", [1, H * R], dt.bfloat16,
                              kind="ExternalInput")
    eu0_d = nc.dram_tensor("eu0", [R, H], dt.float32, kind="ExternalInput")
    ev0_d = nc.dram_tensor("ev0", [R, H], dt.float32, kind="ExternalInput")
    wcat_d = {}
    for s, (Fin, O, _) in enumerate(STAGES):
        if s == 0:
            continue
        # [W concat by head | W@a_dst (H cols) | W@a_src (H cols)]
        wcat_d[s] = nc.dram_tensor(f"W{s}cat", [Fin, H * O + 2 * H],
                                   dt.bfloat16, kind="ExternalInput")
    ident_d = nc.dram_tensor("ident", [128, 128], dt.bfloat16,
                             kind="ExternalInput")
    wlin_d = nc.dram_tensor("wlin", [H * STAGES[2][1], NCLASS], dt.bfloat16,
                            kind="ExternalInput")
    blin_d = nc.dram_tensor("blin", [1, NCLASS], dt.float32, kind="ExternalInput")
    out_d = nc.dram_tensor("out_blk", [R, NCLASS], dt.float32,
                           kind="ExternalOutput")

    # ---- internal DRAM (stage hand-off + collectives) ----
    RC = R // NCH                      # own rows per chunk
    ccin_d, ccout_d = {}, {}
    for s, (Fin, O, _) in enumerate(STAGES):
        if s < 2:
            _, _, Wn = _ext_cols(STAGES[s + 1][1])
            ccin_d[s] = nc.dram_tensor(f"ccin{s}", [R, H * Wn], dt.bfloat16,
                                       kind="Internal")
            for k in range(NCH):
                ccout_d[(s, k)] = nc.dram_tensor(
                    f"ccout{s}_{k}", [NCORES * RC, H * Wn], dt.bfloat16,
                    kind="Internal", addr_space="Shared")

    order = _tile_order()

    with tile.TileContext(nc) as tc:
        with (
            tc.tile_pool(name="glob", bufs=1) as gp,
            tc.tile_pool(name="work", bufs=4) as wp,
            tc.tile_pool(name="small", bufs=2) as sp,
            tc.tile_pool(name="psum", bufs=1, space="PSUM") as pp,
            tc.tile_pool(name="psum2", bufs=2, space="PSUM") as pp2,
        ):
            ones_bf = gp.tile([1, 128], dt.bfloat16, tag="ones_bf")
            nc.gpsimd.memset(ones_bf[:], 1.0)
            ones_f = gp.tile([1, 128], dt.float32, tag="ones_f")
            nc.gpsimd.memset(ones_f[:], 1.0)

            ACC_W = 396  # per-i-chunk PSUM bank: G*(2E) A-sums + G*E m-sums

            # All input-dependent loads live INSIDE the rep loop so each rep
            # is a complete execution (full HBM traffic) — reps>1 exists only
            # to amortize NEFF launch overhead when timing.
            for rep in range(reps):
              # small stage-1 dependencies first so attention starts early
              f1n_sb = gp.tile([1, H, R], dt.bfloat16, tag="f1n_sb")
              nc.scalar.dma_start(f1n_sb[:], f1neg0_d[:].rearrange(
                  "q (h r) -> q h r", h=H))
              eu = gp.tile([128, IC, H], dt.float32, tag="eu")
              ev = gp.tile([128, IC, H], dt.float32, tag="ev")
              nc.scalar.dma_start(
                  eu[:], eu0_d[:].rearrange("(i p) h -> p i h", p=128))
              nc.scalar.dma_start(
                  ev[:], ev0_d[:].rearrange("(i p) h -> p i h", p=128))

              # stage-1 ext rows (host-built) on sync queue, masks on the ACT
              # HWDGE queue — interleaved per tile so tile t's deps co-arrive
              uwx0 = [gp.tile([128, H, W0], dt.bfloat16, tag="uwx0", bufs=NT,
                              name=f"uwx0_{t}") for t in range(NT)]
              mask = [gp.tile([128, R], dt.bfloat16, tag="mask", bufs=NT,
                              name=f"mk_{t}") for t in range(NT)]
              for t in range(NT):
                  nc.sync.dma_start(
                      uwx0[t][:],
                      uext0_d[t * 128:(t + 1) * 128, :].rearrange(
                          "p (h w) -> p h w", h=H))
                  nc.gpsimd.dma_start(mask[t][:], adjT[t * 128:(t + 1) * 128, :])

              wcat_t = {}
              for s, (Fin, O, _) in enumerate(STAGES):
                  if s == 0:
                      continue
                  ft_n = Fin // 128
                  w = gp.tile([128, ft_n, H * O + 2 * H], dt.bfloat16,
                              tag=f"wcat{s}")
                  for ft in range(ft_n):
                      nc.scalar.dma_start(w[:, ft, :],
                                          wcat_d[s][ft * 128:(ft + 1) * 128, :])
                  wcat_t[s] = w
              ident = gp.tile([128, 128], dt.bfloat16, tag="ident")
              nc.scalar.dma_start(ident[:], ident_d[:])
              wlin_t = gp.tile([H * STAGES[2][1], NCLASS], dt.bfloat16,
                               tag="wlin")
              nc.scalar.dma_start(wlin_t[:], wlin_d[:])
              blin_t = gp.tile([1, NCLASS], dt.float32, tag="blin")
              nc.scalar.dma_start(blin_t[:], blin_d[:])

              hT_own = None
              for s, (Fin, O, groups) in enumerate(STAGES):
                  ft_n = Fin // 128
                  HO = H * O
                  E, D, Wd = _ext_cols(O)
                  act_heads = ACT_HEADS_BY_STAGE[s]

                  f1b = gp.tile([128, H, R], dt.bfloat16, tag="f1b")
                  evn = gp.tile([128, IC, H], dt.float32, tag="evn")

                  if s == 0:
                      uwx = uwx0
                      tile_seq = list(range(NT))
                      for h in range(H):
                          f1bps = pp2.tile([128, R], dt.float32, tag="mm_ps",
                                           name="f1bps")
                          nc.tensor.matmul(f1bps[:], ones_bf[:],
                                           f1n_sb[:, h, :], start=True,
                                           stop=True)
                          nc.scalar.activation(f1b[:, h, :], f1bps[:], AF.Copy)
                  else:
                      eu = gp.tile([128, IC, H], dt.float32, tag="eu2",
                                   name=f"eu{s}")
                      ev = gp.tile([128, IC, H], dt.float32, tag="ev2",
                                   name=f"ev{s}")
                      # ---- own-rows ext build, chunked for overlap ----
                      uo = gp.tile([128, NTO, H, Wd], dt.bfloat16, tag="uo",
                                   name=f"uo{s}")
                      whs = gp.tile([128, NTO, H, O], dt.bfloat16, tag="whs",
                                    name=f"whs{s}")
                      f2c = sp.tile([128, NTO, H], dt.float32, tag="f2c")
                      for k in range(NCH):
                          for nt in range(k * TPC, (k + 1) * TPC):
                              ps = pp2.tile([128, HO + 2 * H], dt.float32,
                                            tag="mm_ps", name="wh_ps")
                              for ft in range(ft_n):
                                  nc.tensor.matmul(
                                      ps[:],
                                      hT_own[:, ft, nt * 128:(nt + 1) * 128],
                                      wcat_t[s][:, ft, :],
                                      start=(ft == 0), stop=(ft == ft_n - 1))
                              psv = ps[:, 0:HO].rearrange("p (h o) -> p h o",
                                                          h=H)
                              nc.scalar.activation(whs[:, nt, :, :], psv,
                                                   AF.Copy)
                              nc.scalar.activation(f2c[:, nt, :],
                                                   ps[:, HO:HO + H], AF.Copy)
                          cs = slice(k * TPC, (k + 1) * TPC)
                          nc.scalar.activation(uo[:, cs, :, D:D + 1],
                                               f2c[:, cs], AF.Copy)
                          nc.scalar.activation(uo[:, cs, :, O:O + 1],
                                               f2c[:, cs], AF.Exp)
                          nc.scalar.activation(uo[:, cs, :, D - 1:D],
                                               f2c[:, cs], AF.Exp, scale=0.2)
                          ub = uo[:, cs, :, O:O + 1].broadcast_to(
                              (128, TPC, H, O))
                          nc.vector.tensor_tensor(uo[:, cs, :, 0:O],
                                                  whs[:, cs], ub, OP.mult)
                          vb = uo[:, cs, :, D - 1:D].broadcast_to(
                              (128, TPC, H, O))
                          nc.vector.tensor_tensor(uo[:, cs, :, E:E + O],
                                                  whs[:, cs], vb, OP.mult)
                          nc.vector.memset(uo[:, cs, :, D + 1:D + 2], 0.0)
                          for t in range(k * TPC, (k + 1) * TPC):
                              nc.sync.dma_start(
                                  ccin_d[s - 1][t * 128:(t + 1) * 128, :],
                                  uo[:, t, :, :].rearrange("p h w -> p (h w)"))
                          if single or "noag" in ablate:
                              for c in range(NCORES):
                                  nc.sync.dma_start(
                                      ccout_d[(s - 1, k)][c * RC:(c + 1) * RC, :],
                                      ccin_d[s - 1][k * RC:(k + 1) * RC, :])
                          else:
                              nc.gpsimd.collective_compute(
                                  "AllGather", OP.bypass,
                                  replica_groups=[list(range(NCORES))],
                                  ins=[ccin_d[s - 1][k * RC:(k + 1) * RC, :]],
                                  outs=[ccout_d[(s - 1, k)][:]])

                      uwx = [None] * NT
                      tile_seq = order
                      for i, t in enumerate(order):
                          r, l = t // NTO, t % NTO
                          k, lk = l // TPC, l % TPC
                          uwx[t] = gp.tile([128, H, Wd], dt.bfloat16,
                                           tag="uwx", bufs=NT,
                                           name=f"uwx{s}_{t}")
                          row0 = r * RC + lk * 128
                          nc.sync.dma_start(
                              uwx[t][:],
                              ccout_d[(s - 1, k)][row0:row0 + 128, :].rearrange(
                                  "p (h w) -> p h w", h=H))

                      # ---- f1 (free layout, negated, broadcast over parts)
                      for h in range(H):
                          f1ps = pp2.tile([1, R], dt.float32, tag="mm_ps",
                                          name="f1ps")
                          for ft in range(ft_n):
                              nc.tensor.matmul(
                                  f1ps[:],
                                  wcat_t[s][:, ft, HO + H + h:HO + H + h + 1],
                                  hT_own[:, ft, :],
                                  start=(ft == 0), stop=(ft == ft_n - 1))
                          f1sb = sp.tile([1, R], dt.bfloat16, tag="f1_sb")
                          nc.scalar.copy(f1sb[:], f1ps[:])
                          f1bps = pp2.tile([128, R], dt.float32, tag="mm_ps",
                                           name="f1bps")
                          nc.tensor.matmul(f1bps[:], ones_bf[:], f1sb[:],
                                           start=True, stop=True)
                          nc.scalar.activation(f1b[:, h, :], f1bps[:], AF.Copy,
                                               scale=-1.0)
                      # ---- eu/ev for own i-chunks
                      f1pa = gp.tile([128, IC, H], dt.float32, tag="f1pa")
                      for ic in range(IC):
                          wops = pp2.tile([128, 2 * H], dt.float32, tag="mm_ps",
                                          name="wops")
                          for ft in range(ft_n):
                              nc.tensor.matmul(
                                  wops[:],
                                  hT_own[:, ft, ic * 128:(ic + 1) * 128],
                                  wcat_t[s][:, ft, HO:HO + 2 * H],
                                  start=(ft == 0), stop=(ft == ft_n - 1))
                          nc.scalar.activation(f1pa[:, ic, :], wops[:, H:2 * H],
                                               AF.Copy)
                      nc.scalar.activation(eu[:], f1pa[:], AF.Exp)
                      nc.scalar.activation(ev[:], f1pa[:], AF.Exp, scale=0.2)

                  nc.vector.tensor_scalar_mul(evn[:], ev[:], -1.0)
                  eu2 = gp.tile([128, IC, H], dt.float32, tag="eu2h")
                  nc.vector.tensor_scalar_mul(eu2[:], eu[:], 0.5)
                  ev2 = gp.tile([128, IC, H], dt.float32, tag="ev2h")
                  nc.vector.tensor_scalar_mul(ev2[:], ev[:], 0.5)
                  evn2 = gp.tile([128, IC, H], dt.float32, tag="evn2h")
                  nc.vector.tensor_scalar_mul(evn2[:], ev[:], -0.5)
                  # fp32 copies of the per-j f2 compare scalars (is_lt wants f32)
                  f2s = [sp.tile([128, H], dt.float32, tag="f2s", bufs=NT,
                                 name=f"f2s{s}_{t}") for t in range(NT)]
                  for t in tile_seq:
                      nc.scalar.activation(f2s[t][:], uwx[t][:, :, D], AF.Copy)

                  # ---- attention: fused compare*mask + matmul accumulation
                  hn_tiles = [gp.tile([128, HO], dt.bfloat16, tag=f"hn_{ic}",
                                      name=f"hn{s}_{ic}")
                              for ic in range(IC)]
                  for grp in groups:
                      G = len(grp)
                      # per-head mask-sum width: Act(sign) heads need full D
                      mw = [D if h in act_heads else E for h in grp]
                      moff = [G * D + sum(mw[:gi]) for gi in range(G)]
                      acc_w = G * D + sum(mw)
                      accs = [pp.tile([128, acc_w], dt.float32, tag=f"accAB_{ic}",
                                      name=f"acc{s}_{grp[0]}_{ic}")
                              for ic in range(IC)]
                      # contiguous m-matmul runs of equal per-head width
                      mruns = []
                      for gi, h in enumerate(grp):
                          if mruns and mw[gi] == mruns[-1][2]:
                              mruns[-1][1] += 1
                          else:
                              mruns.append([gi, gi + 1, mw[gi]])
                      if "novec" in ablate:
                          Astat = gp.tile([128, G, R], dt.bfloat16,
                                          tag="Astat", name=f"Astat{s}_{grp[0]}")
                          nc.vector.memset(Astat[:], 1.0)
                      for ti, nt in enumerate(tile_seq):
                          if "novec" in ablate:
                              A = Astat
                          else:
                              cInd = wp.tile([128, G, R], dt.bfloat16,
                                             tag="cInd", bufs=4)
                              for gi, h in enumerate(grp):
                                  if h in act_heads:
                                      nc.scalar.activation(
                                          cInd[:, gi, :], f1b[:, h, :],
                                          AF.Sign,
                                          bias=f2s[nt][:, h:h + 1],
                                          scale=-1.0)
                                  else:
                                      nc.vector.tensor_scalar(
                                          cInd[:, gi, :], f1b[:, h, :],
                                          f2s[nt][:, h:h + 1], None, OP.is_lt)
                              A = wp.tile([128, G, R], dt.bfloat16, tag="A",
                                          bufs=4)
                              mb_ = mask[nt][:, None, :].broadcast_to(
                                  (128, G, R))
                              nc.vector.tensor_tensor(A[:], cInd[:], mb_,
                                                      OP.mult)
                          for gi, h in enumerate(grp):
                              for ic in range(IC):
                                  nc.tensor.matmul(
                                      accs[ic][:, gi * D:(gi + 1) * D],
                                      A[:, gi, ic * 128:(ic + 1) * 128],
                                      uwx[nt][:, h, 0:D],
                                      start=(ti == 0), stop=(ti == NT - 1))
                          for g0, g1, w_ in mruns:
                              c0 = 0 if w_ == D else E
                              for ic in range(IC):
                                  nc.tensor.matmul(
                                      accs[ic][:, moff[g0]:
                                               moff[g0] + (g1 - g0) * w_],
                                      mask[nt][:, ic * 128:(ic + 1) * 128],
                                      uwx[nt][:, grp[0] + g0:grp[0] + g1,
                                              c0:D],
                                      start=(ti == 0), stop=(ti == NT - 1))

                      # ---- epilogue: h = elu((eu*Su + ev*(Mv - Sv)) / Z)
                      # DVE heads: S_A = pa (0/1 selector);
                      # Act heads:  S_A = (pa'' + pm)/2 (sign selector)
                      for ic in range(IC):
                          for gi, h in enumerate(grp):
                              act_h = h in act_heads
                              pa_u = accs[ic][:, gi * D:gi * D + E]
                              pa_v = accs[ic][:, gi * D + E:(gi + 1) * D]
                              d1 = sp.tile([128, E], dt.float32, tag="d1")
                              d2 = sp.tile([128, E], dt.float32, tag="d2")
                              d3 = sp.tile([128, E], dt.float32, tag="d3")
                              if act_h:
                                  pm_u = accs[ic][:, moff[gi]:moff[gi] + E]
                                  pm_v = accs[ic][:, moff[gi] + E:
                                                 moff[gi] + D]
                                  nc.scalar.activation(
                                      d1[:], pa_u, AF.Copy,
                                      scale=eu2[:, ic, h:h + 1])
                                  d0 = sp.tile([128, E], dt.float32, tag="d0")
                                  nc.vector.scalar_tensor_tensor(
                                      d0[:], pm_u, eu2[:, ic, h:h + 1], d1[:],
                                      OP.mult, OP.add)
                                  nc.vector.scalar_tensor_tensor(
                                      d2[:], pm_v, ev2[:, ic, h:h + 1], d0[:],
                                      OP.mult, OP.add)
                                  nc.vector.scalar_tensor_tensor(
                                      d3[:], pa_v, evn2[:, ic, h:h + 1], d2[:],
                                      OP.mult, OP.add)
                              else:
                                  pm = accs[ic][:, moff[gi]:moff[gi] + E]
                                  nc.scalar.activation(
                                      d1[:], pa_u, AF.Copy,
                                      scale=eu[:, ic, h:h + 1])
                                  nc.vector.scalar_tensor_tensor(
                                      d2[:], pm, ev[:, ic, h:h + 1], d1[:],
                                      OP.mult, OP.add)
                                  nc.vector.scalar_tensor_tensor(
                                      d3[:], pa_v, evn[:, ic, h:h + 1], d2[:],
                                      OP.mult, OP.add)
                              r = sp.tile([128, 1], dt.float32, tag="rZ")
                              nc.vector.reciprocal(r[:], d3[:, O:O + 1])
                              t0 = sp.tile([128, O], dt.float32, tag="t0")
                              nc.vector.tensor_scalar(t0[:], d3[:, 0:O], r[:],
                                                      0.0, OP.mult, OP.min)
                              t1 = sp.tile([128, O], dt.float32, tag="t1")
                              nc.vector.tensor_scalar(t1[:], d3[:, 0:O], r[:],
                                                      0.0, OP.mult, OP.max)
                              e0 = sp.tile([128, O], dt.float32, tag="e0")
                              nc.scalar.activation(e0[:], t0[:], AF.Exp)
                              nc.vector.scalar_tensor_tensor(
                                  hn_tiles[ic][:, h * O:(h + 1) * O], e0[:], 1.0,
                                  t1[:], OP.subtract, OP.add)

                  # ---- hand-off: PE-transpose own rows for next stage ----
                  if s < 2:
                      nft = HO // 128
                      hT_own = gp.tile([128, nft, R], dt.bfloat16, tag="hTown",
                                       name=f"hTown{s + 1}")
                      for ic in range(IC):
                          for ft in range(nft):
                              tp = pp2.tile([128, 128], dt.bfloat16,
                                            tag="mm_ps", name="tp_ps")
                              nc.tensor.transpose(
                                  tp[:], hn_tiles[ic][:, ft * 128:(ft + 1) * 128],
                                  ident[:])
                              nc.scalar.activation(
                                  hT_own[:, ft, ic * 128:(ic + 1) * 128], tp[:],
                                  AF.Copy)

              # ---- final linear + log_softmax ----
              F3 = H * STAGES[2][1]  # 64
              h3T = gp.tile([F3, R], dt.bfloat16, tag="h3T")
              for ic in range(IC):
                  tp = pp2.tile([128, 128], dt.bfloat16, tag="mm_ps",
                                name=f"tp3_{ic}")
                  nc.tensor.transpose(tp[:F3, :], hn_tiles[ic][:, 0:F3],
                                      ident[:])
                  nc.scalar.activation(h3T[:, ic * 128:(ic + 1) * 128],
                                       tp[:F3, :], AF.Copy)

              blb_ps = pp2.tile([128, NCLASS], dt.float32, tag="mm_ps",
                                name="blb_ps")
              nc.tensor.matmul(blb_ps[:], ones_f[:], blin_t[:], start=True,
                               stop=True)
              blb = gp.tile([128, NCLASS], dt.float32, tag="blb")
              nc.vector.tensor_copy(blb[:], blb_ps[:])

              for ic in range(IC):
                  lg_ps = pp2.tile([128, NCLASS], dt.float32, tag="mm_ps",
                                   name="lg_ps")
                  nc.tensor.matmul(lg_ps[:], h3T[:, ic * 128:(ic + 1) * 128],
                                   wlin_t[:], start=True, stop=True)
                  lg = sp.tile([128, NCLASS], dt.float32, tag="lg")
                  nc.vector.tensor_tensor(lg[:], lg_ps[:], blb[:], OP.add)
                  mx = sp.tile([128, 1], dt.float32, tag="mx")
                  nc.vector.tensor_reduce(mx[:], lg[:], axis=X, op=OP.max)
                  negmx = sp.tile([128, 1], dt.float32, tag="negmx")
                  nc.vector.tensor_scalar_mul(negmx[:], mx[:], -1.0)
                  ex = sp.tile([128, NCLASS], dt.float32, tag="ex")
                  se = sp.tile([128, 1], dt.float32, tag="se")
                  nc.scalar.activation(ex[:], lg[:], AF.Exp, bias=negmx[:],
                                       accum_out=se[:])
                  ln_t = sp.tile([128, 1], dt.float32, tag="ln_t")
                  nc.scalar.activation(ln_t[:], se[:], AF.Ln)
                  negln = sp.tile([128, 1], dt.float32, tag="negln")
                  nc.vector.tensor_scalar_mul(negln[:], ln_t[:], -1.0)
                  ov = sp.tile([128, NCLASS], dt.float32, tag="ov")
                  nc.vector.tensor_scalar(ov[:], lg[:], negmx[:], negln[:],
                                          OP.add, OP.add)
                  nc.sync.dma_start(out_d[ic * 128:(ic + 1) * 128, :], ov[:])

    nc.compile()
    return nc


def _get_nc():
    if "nc" not in _CACHE:
        _CACHE["nc"] = _build()
    return _CACHE["nc"]


def _prep_in_maps(x, adj, W1, a1, W2, a2, W3, a3, Wlin, blin):
    import ml_dtypes
    bf16 = ml_dtypes.bfloat16

    x = np.asarray(x, np.float32)
    # transpose adj once (BLAS-friendly), slice per-core column blocks below
    adjT_bf = np.ascontiguousarray(
        (np.asarray(adj) > 0).T).astype(bf16)

    Ws = [np.asarray(W1, np.float32), np.asarray(W2, np.float32),
          np.asarray(W3, np.float32)]
    As = [np.asarray(a1, np.float32), np.asarray(a2, np.float32),
          np.asarray(a3, np.float32)]

    # ---- host-side stage-1 prep (exact fp32, BLAS matmuls) ----
    O0 = STAGES[0][1]
    E0, D0, W0c = _ext_cols(O0)
    Wh1 = (x @ Ws[0].transpose(1, 0, 2).reshape(F0, H * O0)).reshape(
        N, H, O0)                                                # [N,H,O]
    f2_1 = np.einsum('nho,ho->nh', Wh1, As[0][:, O0:])
    f1_1 = np.einsum('nho,ho->nh', Wh1, As[0][:, :O0])
    u1 = np.exp(f2_1)
    v1 = np.exp(0.2 * f2_1)
    uext0 = np.zeros((N, H, W0c), np.float32)
    uext0[:, :, 0:O0] = u1[:, :, None] * Wh1
    uext0[:, :, O0] = u1
    uext0[:, :, E0:E0 + O0] = v1[:, :, None] * Wh1
    uext0[:, :, D0 - 1] = v1
    uext0[:, :, D0] = f2_1

    shared = {"uext0": np.ascontiguousarray(
        uext0.reshape(N, H * W0c)).astype(bf16)}
    for s, (Fin, O, _) in enumerate(STAGES):
        if s == 0:
            continue
        W = Ws[s]  # [H, Fin, O]
        a = As[s]  # [H, 2*O]
        wcat = W.transpose(1, 0, 2).reshape(Fin, H * O)
        wd = np.einsum('hfo,ho->fh', W, a[:, O:])   # W @ a_dst
        ws_ = np.einsum('hfo,ho->fh', W, a[:, :O])  # W @ a_src
        shared[f"W{s}cat"] = np.ascontiguousarray(
            np.concatenate([wcat, wd, ws_], axis=1)).astype(bf16)
    shared["ident"] = np.eye(128, dtype=np.float32).astype(bf16)
    shared["wlin"] = np.asarray(Wlin, np.float32).astype(bf16)
    shared["blin"] = np.asarray(blin, np.float32).reshape(1, NCLASS)

    in_maps = []
    for c in range(NCORES):
        rows = slice(c * R, (c + 1) * R)
        m = dict(shared)
        m["adjT"] = np.ascontiguousarray(adjT_bf[:, rows])
        m["f1neg0"] = np.ascontiguousarray(
            (-f1_1[rows, :]).T.reshape(1, H * R)).astype(bf16)
        m["eu0"] = np.ascontiguousarray(np.exp(f1_1[rows, :]))
        m["ev0"] = np.ascontiguousarray(np.exp(0.2 * f1_1[rows, :]))
        in_maps.append(m)
    return in_maps


def _make_runner(nc):
    """Cached shard_map'd executable (same lowering as
    concourse.bass2jax.run_bass_via_pjrt, traced once; repeated kernel()
    calls skip re-trace and only pay H2D + execute + D2H)."""
    import jax
    from jax.sharding import Mesh, PartitionSpec
    from jax.experimental.shard_map import shard_map
    import concourse.bass2jax as b2j
    import concourse.mybir as mybir

    b2j.install_neuronx_cc_hook()
    partition_name = (nc.partition_id_tensor.name
                      if nc.partition_id_tensor else None)
    in_names, out_names, out_avals = [], [], []
    for alloc in nc.m.functions[0].allocations:
        if not isinstance(alloc, mybir.MemoryLocationSet):
            continue
        name = alloc.memorylocations[0].name
        if alloc.kind == "ExternalInput":
            if name != partition_name:
                in_names.append(name)
        elif alloc.kind == "ExternalOutput":
            out_avals.append(jax.core.ShapedArray(
                tuple(alloc.tensor_shape), mybir.dt.np(alloc.dtype)))
            out_names.append(name)
    n_params = len(in_names)
    all_in = list(in_names) + list(out_names)
    if partition_name:
        all_in.append(partition_name)

    def _body(*args):
        operands = list(args)
        if partition_name:
            operands.append(b2j.partition_id_tensor())
        outs = b2j._bass_exec_p.bind(
            *operands, out_avals=tuple(out_avals), in_names=tuple(all_in),
            out_names=tuple(out_names), lowering_input_output_aliases=(),
            sim_require_finite=True, sim_require_nnan=True, nc=nc)
        return tuple(outs)

    devices = jax.devices()[:NCORES]
    mesh = Mesh(np.asarray(devices), ("core",))
    n_outs = len(out_names)
    in_specs = (PartitionSpec("core"),) * (n_params + n_outs)
    out_specs = (PartitionSpec("core"),) * n_outs
    fn = jax.jit(shard_map(_body, mesh=mesh, in_specs=in_specs,
                           out_specs=out_specs, check_rep=False),
                 keep_unused=True)
    zero_outs = [np.zeros((NCORES * av.shape[0], *av.shape[1:]), av.dtype)
                 for av in out_avals]

    def run(in_maps):
        concat_in = [np.concatenate([np.asarray(in_maps[c][name])
                                     for c in range(NCORES)], axis=0)
                     for name in in_names]
        outs = fn(*concat_in, *zero_outs)
        i = out_names.index("out_blk")
        return np.asarray(outs[i])  # [NCORES*R, NCLASS] row-concat

    return run


def kernel(x, adj, W1, a1, W2, a2, W3, a3, Wlin, blin):
    nc = _get_nc()
    if "run" not in _CACHE:
        _CACHE["run"] = _make_runner(nc)
    in_maps = _prep_in_maps(x, adj, W1, a1, W2, a2, W3, a3, Wlin, blin)
    out = _CACHE["run"](in_maps)
    return np.ascontiguousarray(out).astype(np.float32)
